# revision 1
# baseline (speedup 1.0000x reference)
"""BarrierNet (MLP 4->512->{128,128}->{2,2} + closed-form QP) on 8 Trainium2 cores.

Data-parallel: batch 262144 sharded 8 x 32768; weights replicated.

Per-core layout: sample s = p*256 + f (p = SBUF partition, f = 0..256).
The MLP runs feature-major (batch on the PE free dim, weights stationary):
the host pre-transposes x into xT5 [5, 32768] (ones row folds b1 into the
L1 matmul) tiled so tile t covers f in [4t, 4t+4) with column c = j*128 + p.
xT5 is staged once into a [128, 8192] SBUF tile at partition bases
{0,32,64,96} (tile t at base 32*(t%4)) so the L1 row-packed matmuls read it
directly. L2 contracts K=512 in 4 chunks; L3 computes (x31 | z32) stacked as
[4, 512] matmuls batched 2 tiles per PSUM pair. Those rows are staged into
S [128, 512] per 32 tiles, PE-transposed to sample-major, and the QP
(sin/cos via range-reduced ACT Sin, sigmoid, one reciprocal) runs as wide
[128, 256] vector ops reading px/py/theta/v straight from x, which is
already sample-major in DRAM. All matmuls use float32r (full PE rate).
"""
import numpy as np
from contextlib import ExitStack

import concourse.bass as bass
from concourse import bacc as bacc_mod
import concourse.tile as tile
from concourse import mybir
from concourse.bass_utils import run_bass_kernel_spmd

F32 = mybir.dt.float32
F32R = mybir.dt.float32r
AF = mybir.ActivationFunctionType
OP = mybir.AluOpType

MAGIC = float(np.float32(1.5 * 2 ** 23))
INV2PI = float(np.float32(1.0 / (2 * np.pi)))
TWOPI = float(np.float32(2 * np.pi))
HALFPI = float(np.float32(np.pi / 2))

N_CORES = 8
NB = 262144
B = NB // N_CORES  # 32768 per core

_CACHE = {}


def _emit(nc, tc, ctx, aps, B, reps=1, dbg=None):
    (xT5, x_nat, w1r, w2s, w3s, b2s, b3bc, ident, u_out) = aps
    T = B // 512
    NS = T // 32
    F = B // 128

    const = ctx.enter_context(tc.tile_pool(name="const", bufs=1))
    lp = ctx.enter_context(tc.tile_pool(name="lp", bufs=1))
    qp = ctx.enter_context(tc.tile_pool(name="qp", bufs=1))
    ps = ctx.enter_context(tc.tile_pool(name="ps", bufs=1, space="PSUM"))
    dr = ctx.enter_context(tc.tile_pool(name="dr", bufs=1, space="DRAM"))

    w1_sb = const.tile([128, 512], F32R, name="w1_sb", tag="w1_sb")
    nc.sync.dma_start(w1_sb[:], w1r[:])
    w2_sb = const.tile([128, 8, 128], F32R, name="w2_sb", tag="w2_sb")
    nc.sync.dma_start(w2_sb[:], w2s[:])
    w3_sb = const.tile([128, 2, 4], F32R, name="w3_sb", tag="w3_sb")
    nc.sync.dma_start(w3_sb[:], w3s[:])
    b2_sb = const.tile([128, 2], F32, name="b2_sb", tag="b2_sb")
    nc.sync.dma_start(b2_sb[:], b2s[:])
    b3_sb = const.tile([128, 4], F32, name="b3_sb", tag="b3_sb")
    nc.sync.dma_start(b3_sb[:], b3bc[:])
    id_sb = const.tile([128, 128], F32, name="id_sb", tag="id_sb")
    nc.sync.dma_start(id_sb[:], ident[:])
    xn_sb = const.tile([128, F, 4], F32, name="xn_sb", tag="xn_sb")
    nc.sync.dma_start(xn_sb[:], x_nat[:])
    # X4: tile t lives at partition base 32*(t%4), columns 512*(t//4)..
    x4_sb = const.tile([128, T // 4, 512], F32R, name="x4_sb", tag="x4_sb")
    xv = xT5.rearrange("q (tt c r) -> q tt c r", c=4, r=512)
    for c in range(4):
        nc.sync.dma_start(
            x4_sb[32 * c:32 * c + 5, :, :], xv[:, :, c, :])
    # T': free idx = sp*512 + j*128 + q*32 + g
    tp_sb = const.tile([128, NS, 4, 4, 32], F32, name="tp_sb", tag="tp_sb")

    for rep in range(reps):
        # software-pipelined: window w runs L1(w), L2(w-1), L3(w-2)
        h1s = {}
        h2s = {}
        s_sbs = {}
        geo = {}
        stg = None
        finalize = {}
        for w in range(T + 3):
            for sp_fin, fn in list(finalize.items()):
                if w >= sp_fin:
                    fn()
                    del finalize[sp_fin]
            if w < T:
                t = w
                cb = 32 * (t % 4)
                xc = x4_sb[cb:cb + 5, t // 4, :]
                h1 = lp.tile([128, 4, 512], F32R, name="h1", tag="h1", bufs=3)
                h1s[t] = h1
                for fh in range(2):
                    ps1 = ps.tile([128, 2, 512], F32, name="ps1", tag="ps1",
                                  bufs=2)
                    for f2 in range(2):
                        f = 2 * fh + f2
                        nc.tensor.matmul(
                            ps1[:, f2, :],
                            w1_sb[cb:cb + 5, 128 * f:128 * (f + 1)],
                            xc,
                            start=True, stop=True, tile_position=(cb, 0))
                    nc.scalar.activation(h1[:, 2 * fh:2 * fh + 2, :],
                                         ps1[:], AF.Relu)
            if 1 <= w <= T:
                t = w - 1
                h1 = h1s.pop(t)
                ps21 = ps.tile([128, 512], F32, name="ps21", tag="ps21",
                               bufs=1)
                ps22 = ps.tile([128, 512], F32, name="ps22", tag="ps22",
                               bufs=1)
                for i in range(4):
                    nc.tensor.matmul(ps21[:], w2_sb[:, i, :], h1[:, i, :],
                                     start=(i == 0), stop=(i == 3))
                for i in range(4):
                    nc.tensor.matmul(ps22[:], w2_sb[:, 4 + i, :], h1[:, i, :],
                                     start=(i == 0), stop=(i == 3))
                h21 = lp.tile([128, 512], F32R, name="h21", tag="h21", bufs=3)
                nc.vector.tensor_scalar(h21[:], ps21[:], b2_sb[:, 0:1],
                                        0.0, op0=OP.add, op1=OP.max)
                h22 = lp.tile([128, 512], F32R, name="h22", tag="h22", bufs=3)
                nc.vector.tensor_scalar(h22[:], ps22[:], b2_sb[:, 1:2],
                                        0.0, op0=OP.add, op1=OP.max)
                h2s[t] = (h21, h22)
            if 2 <= w <= T + 1:
                t = w - 2
                sp, g = t // 32, t % 32
                if g == 0:
                    s_sbs[sp] = lp.tile([128, 512], F32, name="s_sb",
                                        tag="s_sb", bufs=2)
                    # DRAM bounce buffer in (g, q) order
                    s_sbs[(sp, "dr")] = dr.tile([32, 4, 512], F32,
                                                name="s_dr", tag="s_dr",
                                                bufs=2)
                    geo[sp] = _qp_geo(nc, qp, xn_sb, b3_sb, sp, NS)
                ps3 = ps.tile([4, 512], F32, name="ps3", tag="ps3",
                              bufs=2)
                if g % 8 == 0:
                    stg = lp.tile([4, 8, 512], F32, name="stg", tag="stg",
                                  bufs=3)
                h21, h22 = h2s.pop(t)
                nc.tensor.matmul(ps3[:], w3_sb[:, 0, :], h21[:],
                                 start=True, stop=False)
                nc.tensor.matmul(ps3[:], w3_sb[:, 1, :], h22[:],
                                 start=False, stop=True)
                nc.vector.tensor_copy(stg[:, g % 8, :], ps3[:])
                if g % 8 == 7:
                    g0 = g - 7
                    s_dr = s_sbs[(sp, "dr")]
                    # dst (q, g, n) iteration into (g, q) DRAM layout
                    dstv = s_dr[:].rearrange("g q n -> q g n")[:, g0:g0 + 8, :]
                    nc.sync.dma_start(dstv, stg[:])
                    # quarter reload: rows 4g+q contiguous
                    srcv = s_dr[g0:g0 + 8].rearrange("g q n -> (g q) n")
                    nc.sync.dma_start(s_sbs[sp][4 * g0:4 * g0 + 32, :], srcv)
                if g == 31:
                    finalize[w + 3] = _mk_finalize(
                        nc, qp, ps, lp, s_sbs.pop(sp), id_sb, tp_sb, xn_sb,
                        b3_sb, u_out, sp, NS, F, geo)
                    s_sbs.pop((sp, "dr"))
        for sp_fin, fn in sorted(finalize.items()):
            fn()

def _qp_tile(nc, qp, name, shape=(128, 4, 32)):
    return qp.tile(list(shape), F32, name=name, tag=name, bufs=1)


def _qp_geo(nc, qp, xn_sb, b3_sb, sp, NS):
    """x-only QP quantities (no T' dependency) - run early, under compute."""
    def tt(name, a, b, op):
        o = _qp_tile(nc, qp, name)
        nc.vector.tensor_tensor(o[:], a, b, op=op)
        return o[:]

    def ts(name, a, s1, op0, s2=None, op1=None):
        o = _qp_tile(nc, qp, name)
        if s2 is None:
            nc.vector.tensor_scalar(o[:], a, s1, None, op0=op0)
        else:
            nc.vector.tensor_scalar(o[:], a, s1, s2, op0=op0, op1=op1)
        return o[:]

    def act(name, a, func, bias=0.0):
        o = _qp_tile(nc, qp, name)
        nc.scalar.activation(o[:], a, func, bias=bias)
        return o[:]

    xnv = xn_sb[:].rearrange("p (s g j) f -> p s j g f", s=NS, g=32, j=4)
    PX = xnv[:, sp, :, :, 0]
    PY = xnv[:, sp, :, :, 1]
    TH = xnv[:, sp, :, :, 2]
    VV = xnv[:, sp, :, :, 3]

    dx = ts("dx", PX, 10.0, OP.mult, 10.0, OP.add)
    dy = ts("dy", PY, 10.0, OP.mult, 5.0, OP.add)
    v = ts("v", VV, 2.0, OP.mult, 5.0, OP.add)
    f1 = ts("f1", TH, INV2PI, OP.mult, MAGIC, OP.add)
    f2 = ts("f2", f1, MAGIC, OP.subtract, TWOPI, OP.mult)
    thr = tt("thr", TH, f2, OP.subtract)
    st = act("st", thr, AF.Sin)
    c1 = ts("c1", TH, INV2PI, OP.mult, 0.25, OP.add)
    c2 = ts("c2", c1, MAGIC, OP.add, MAGIC, OP.subtract)
    c3 = ts("c3", c2, TWOPI, OP.mult, HALFPI, OP.subtract)
    thc = tt("thc", TH, c3, OP.subtract)
    ct = act("ct", thc, AF.Sin)
    vst = tt("vst", v, st, OP.mult)
    vct = tt("vct", v, ct, OP.mult)
    dx2 = tt("dx2", dx, dx, OP.mult)
    dy2 = tt("dy2", dy, dy, OP.mult)
    bar = tt("bar", dx2, dy2, OP.add)
    bar16 = ts("bar16", bar, 16.0, OP.mult, 576.0, OP.subtract)
    a3 = tt("a3", dx, vct, OP.mult)
    a4 = tt("a4", dy, vst, OP.mult)
    a5 = tt("a5", a3, a4, OP.add)
    bdot4 = ts("bdot4", a5, 8.0, OP.mult)
    v2 = tt("v2", v, v, OP.mult)
    v22 = ts("v22", v2, 2.0, OP.mult)
    g1a = tt("g1a", dx, vst, OP.mult)
    g1b = tt("g1b", dy, vct, OP.mult)
    G1p = tt("G1p", g1a, g1b, OP.subtract)
    g2a = tt("g2a", dx, ct, OP.mult)
    g2b = tt("g2b", dy, st, OP.mult)
    G2pp = tt("G2pp", g2a, g2b, OP.add)
    q1 = tt("q1", G1p, G1p, OP.mult)
    q2 = tt("q2", G2pp, G2pp, OP.mult)
    q3 = tt("q3", q1, q2, OP.add)
    ggc = ts("ggc", q3, 4.0, OP.mult, 1e-12, OP.max)
    rec = _qp_tile(nc, qp, "rec")
    nc.vector.reciprocal(rec[:], ggc)
    return dict(bar16=bar16, bdot4=bdot4, v22=v22, G1p=G1p, G2pp=G2pp,
                rec=rec[:])


def _mk_finalize(nc, qp, ps, lp, s_sb, id_sb, tp_sb, xn_sb, b3_sb, u_out,
                 sp, NS, F, geo):
    def fin():
        for jj in range(2):
            pst = ps.tile([128, 2, 512], F32, name="pst", tag="ps1", bufs=2)
            for j2 in range(2):
                j = 2 * jj + j2
                nc.tensor.transpose(pst[:, j2, 0:128],
                                    s_sb[:, 128 * j:128 * (j + 1)], id_sb[:])
            # in (j2, r=(g,q)) -> out j*128 + q*32 + g
            dstv = tp_sb[:, sp, 2 * jj:2 * jj + 2, :, :].rearrange(
                "p j q g -> p j g q")
            nc.vector.tensor_copy(dstv, pst[:, :, 0:128])
        _qp_rest(nc, qp, tp_sb, b3_sb, u_out, sp, NS, F, geo.pop(sp))
    return fin


def _qp_rest(nc, qp, tp_sb, b3_sb, u_out, sp, NS, F, g):
    def tt(name, a, b, op):
        o = _qp_tile(nc, qp, name)
        nc.vector.tensor_tensor(o[:], a, b, op=op)
        return o[:]

    def ts(name, a, s1, op0, s2=None, op1=None):
        o = _qp_tile(nc, qp, name)
        if s2 is None:
            nc.vector.tensor_scalar(o[:], a, s1, None, op0=op0)
        else:
            nc.vector.tensor_scalar(o[:], a, s1, s2, op0=op0, op1=op1)
        return o[:]

    def act(name, a, func, bias=0.0):
        o = _qp_tile(nc, qp, name)
        nc.scalar.activation(o[:], a, func, bias=bias)
        return o[:]

    X31A = tp_sb[:, sp, :, 0, :]
    X31B = tp_sb[:, sp, :, 1, :]
    Z32A = tp_sb[:, sp, :, 2, :]
    Z32B = tp_sb[:, sp, :, 3, :]

    sa = act("sa", Z32A, AF.Sigmoid, bias=b3_sb[:, 2:3])
    sb_ = act("sb_", Z32B, AF.Sigmoid, bias=b3_sb[:, 3:4])
    ssum = tt("ssum", sa, sb_, OP.add)
    sprod = tt("sprod", sa, sb_, OP.mult)
    m1 = tt("m1", ssum, g["bdot4"], OP.mult)
    m2 = tt("m2", sprod, g["bar16"], OP.mult)
    m3 = tt("m3", m1, m2, OP.add)
    h = tt("h", g["v22"], m3, OP.add)
    xa = ts("xa", X31A, b3_sb[:, 0:1], OP.add)
    xb = ts("xb", X31B, b3_sb[:, 1:2], OP.add)
    n1 = tt("n1", g["G1p"], xa, OP.mult)
    n2 = tt("n2", g["G2pp"], xb, OP.mult)
    n3 = tt("n3", n1, n2, OP.subtract)
    nn = ts("nn", n3, -2.0, OP.mult)
    num = tt("num", nn, h, OP.subtract)
    numc = ts("numc", num, 0.0, OP.max)
    lam = tt("lam", numc, g["rec"], OP.mult)
    p1 = tt("p1", lam, g["G1p"], OP.mult)
    z1 = ts("z1", p1, -2.0, OP.mult)
    p2 = tt("p2", lam, g["G2pp"], OP.mult)
    z2 = ts("z2", p2, 2.0, OP.mult)

    u_sb = qp.tile([128, 128, 2], F32, name="u_sb", tag="u_sb", bufs=2)
    usv = u_sb[:].rearrange("p (g j) c -> p j g c", g=32, j=4)
    nc.vector.tensor_tensor(usv[:, :, :, 0], z1, xa, op=OP.subtract)
    nc.vector.tensor_tensor(usv[:, :, :, 1], z2, xb, op=OP.subtract)
    nc.sync.dma_start(u_out[:, 128 * sp:128 * (sp + 1), :], u_sb[:])


def _build_kernel(n_cores, B, reps=1):
    nc = bacc_mod.Bacc("TRN2", target_bir_lowering=False, debug=False,
                       num_devices=n_cores)
    T = B // 512
    F = B // 128
    xT5 = nc.dram_tensor("xT5", [5, B], F32R, kind="ExternalInput").ap()
    x_nat = nc.dram_tensor("x_nat", [128, F, 4], F32, kind="ExternalInput").ap()
    w1r = nc.dram_tensor("w1r", [128, 512], F32R, kind="ExternalInput").ap()
    w2s = nc.dram_tensor("w2s", [128, 8, 128], F32R, kind="ExternalInput").ap()
    w3s = nc.dram_tensor("w3s", [128, 2, 4], F32R, kind="ExternalInput").ap()
    b2s = nc.dram_tensor("b2s", [128, 2], F32, kind="ExternalInput").ap()
    b3bc = nc.dram_tensor("b3bc", [128, 4], F32, kind="ExternalInput").ap()
    ident = nc.dram_tensor("ident", [128, 128], F32, kind="ExternalInput").ap()
    u_out = nc.dram_tensor("u_out", [128, F, 2], F32, kind="ExternalOutput").ap()
    aps = (xT5, x_nat, w1r, w2s, w3s, b2s, b3bc, ident, u_out)
    with tile.TileContext(nc) as tc:
        with ExitStack() as ctx:
            _emit(nc, tc, ctx, aps, B, reps=reps)
    nc.compile()
    return nc


def _prep_core_inputs(x_shard, W1, b1, W21, b21, W22, b22, W31, b31, W32, b32):
    Bc = x_shard.shape[0]
    xs = np.ascontiguousarray(x_shard, dtype=np.float32).reshape(
        128, Bc // 512, 4, 4)  # [p, t, j, feat]
    xT5 = np.empty((5, Bc), dtype=np.float32)
    xT5[:4] = xs.transpose(3, 1, 2, 0).reshape(4, Bc)
    xT5[4] = 1.0

    w1r = np.zeros((128, 512), dtype=np.float32)
    w1e = np.concatenate([W1.T, b1[None, :]], axis=0)
    for c in range(4):
        w1r[32 * c:32 * c + 5, :] = w1e

    w2s = np.empty((128, 8, 128), dtype=np.float32)
    w2s[:, 0:4, :] = W21.T.reshape(4, 128, 128).transpose(1, 0, 2)
    w2s[:, 4:8, :] = W22.T.reshape(4, 128, 128).transpose(1, 0, 2)

    w3s = np.zeros((128, 2, 4), dtype=np.float32)
    w3s[:, 0, 0:2] = W31.T
    w3s[:, 1, 2:4] = W32.T

    b2s = np.stack([b21, b22], axis=1).astype(np.float32)
    b3bc = np.tile(np.concatenate([b31, b32])[None, :].astype(np.float32),
                   (128, 1))
    ident = np.eye(128, dtype=np.float32)

    return {
        "xT5": xT5,
        "x_nat": np.ascontiguousarray(x_shard, dtype=np.float32).reshape(
            128, Bc // 128, 4),
        "w1r": w1r,
        "w2s": w2s,
        "w3s": w3s,
        "b2s": b2s,
        "b3bc": b3bc,
        "ident": ident,
    }


def kernel(x, W1, b1, W21, b21, W22, b22, W31, b31, W32, b32, sgn=None):
    x = np.asarray(x, dtype=np.float32)
    args = [np.asarray(a, dtype=np.float32)
            for a in (W1, b1, W21, b21, W22, b22, W31, b31, W32, b32)]

    if "nc" not in _CACHE:
        _CACHE["nc"] = _build_kernel(N_CORES, B)
    nc = _CACHE["nc"]

    in_maps = []
    for c in range(N_CORES):
        shard = x[c * B:(c + 1) * B]
        in_maps.append(_prep_core_inputs(shard, *args))

    res = run_bass_kernel_spmd(nc, in_maps, core_ids=list(range(N_CORES)))
    out = np.empty((NB, 2), dtype=np.float32)
    for c in range(N_CORES):
        out[c * B:(c + 1) * B] = res.results[c]["u_out"].reshape(B, 2)
    return out

